# revision 1
# baseline (speedup 1.0000x reference)
"""CrossAttention Trainium2 kernel (batch-parallel over 8 NeuronCores).

Math (per batch element b):
    q  = Wq  @ xq + bq            [C, N]      (C=256, N=56*56=3136)
    kv = Wkv @ xkv + bkv; k, v = split(kv)
    S[n, m]  = q[:, n] . k[:, m]
    denom[m] = ||q[:, m]|| * ||k[:, m]|| + eps      (torch-broadcast quirk:
               divides along the LAST axis m, same index for both norms)
    A = softmax(S / denom, axis=m)
    out = Wproj @ (A @ v^T)^T + bproj  -> reshape + x_q residual

Device mapping (one batch element per core):
  * Everything is computed transposed where it helps:
      S^T[m, n] tiles (m on partitions) make 1/denom[m] a native per-partition
      activation scale, so exp(S*scale) is ONE fused ACT op per tile.
      |S/denom| <= 1 by Cauchy-Schwarz, so softmax needs no max-subtraction.
  * Wproj is folded into v on the host: pv = (Wproj @ Wv) @ xkv. The AV matmul
    then directly produces projected outputs; bias terms fold to
    bo = Wproj @ bv + bproj added at the end (softmax rows sum to 1).
  * AV uses an augmented pv^T|1 moving operand so the softmax row-sum arrives
    as output channel 256 of the same matmuls (no separate reduction).
  * All matmuls run in float32r (TF32-like, full PE rate); producers round
    explicitly (DVE/GPSIMD copies) as the ISA requires.
  * Column norms ||q[:, n]|| are computed from transposed projection tiles
    (qT = xq^T WqT) with ACT Square+accum_out (free-axis reduction).
"""

import sys

if "/opt/trn_rl_repo" not in sys.path:
    sys.path.insert(0, "/opt/trn_rl_repo")

import numpy as np

import concourse.bass as bass
import concourse.mybir as mybir
import concourse.tile as tile
from concourse import bacc
from concourse.bass_utils import run_bass_kernel_spmd
from concourse.masks import make_identity
from contextlib import ExitStack

F32 = mybir.dt.float32
F32R = mybir.dt.float32r
AF = mybir.ActivationFunctionType

P = 128
C = 256
CC = C // P          # 2 channel chunks
N = 56 * 56          # 3136
EPS = 1e-6
NT = 512             # free-dim tile for S^T / projections
N_TILES = [(i, min(NT, N - i)) for i in range(0, N, NT)]          # 7 tiles
M_CHUNKS = [(i, min(P, N - i)) for i in range(0, N, P)]           # 25 chunks


def _mm(nc, out, lhsT, rhs, start, stop):
    nc.tensor.matmul(out, lhsT, rhs, start=start, stop=stop)


def build(use_bias: bool, bench_reps: int = 0):
    nc = bacc.Bacc(None, target_bir_lowering=False)

    xq_d = nc.dram_tensor("xq", [C, N], F32, kind="ExternalInput")
    xkv_d = nc.dram_tensor("xkv", [C, N], F32, kind="ExternalInput")
    wq_d = nc.dram_tensor("wqT", [C, C], F32, kind="ExternalInput")   # Wq.T
    wk_d = nc.dram_tensor("wkT", [C, C], F32, kind="ExternalInput")   # Wk.T
    w3_d = nc.dram_tensor("w3T", [C, C], F32, kind="ExternalInput")   # (Wproj@Wv).T
    bq_d = nc.dram_tensor("bq", [C], F32, kind="ExternalInput")
    bk_d = nc.dram_tensor("bk", [C], F32, kind="ExternalInput")
    bo_d = nc.dram_tensor("bo", [C], F32, kind="ExternalInput")       # Wproj@bv+bproj
    out_d = nc.dram_tensor("out", [C, N], F32, kind="ExternalOutput")

    xq_v = xq_d[:].rearrange("(cc p) n -> p cc n", p=P)
    xkv_v = xkv_d[:].rearrange("(cc p) n -> p cc n", p=P)
    out_v = out_d[:].rearrange("(cc p) n -> p cc n", p=P)

    with tile.TileContext(nc) as tc, ExitStack() as ctx:
        # ---------- persistent pools ----------
        pers = ctx.enter_context(tc.tile_pool(name="pers", bufs=1))
        small = ctx.enter_context(tc.tile_pool(name="small", bufs=2))
        mm512 = ctx.enter_context(tc.tile_pool(name="mm512", bufs=3, space="PSUM"))
        accp = ctx.enter_context(tc.tile_pool(name="accp", bufs=4, space="PSUM"))

        xq_r = pers.tile([P, CC, N], F32R)
        xkv_r = pers.tile([P, CC, N], F32R)
        q_r = pers.tile([P, CC, N], F32R)
        k_r = pers.tile([P, CC, N], F32R)
        pvT = pers.tile([P, len(M_CHUNKS), C + 2], F32R)
        wq_r = pers.tile([P, CC, C], F32R)
        wk_r = pers.tile([P, CC, C], F32R)
        w3_r = pers.tile([P, CC, C], F32R)
        ident = pers.tile([P, P], F32)
        qn2 = pers.tile([P, len(M_CHUNKS)], F32)
        kn2 = pers.tile([P, len(M_CHUNKS)], F32)
        rd = pers.tile([P, len(M_CHUNKS)], F32)
        bq_sb = pers.tile([P, CC], F32)
        bk_sb = pers.tile([P, CC], F32)
        bo_sb = pers.tile([P, CC], F32)
        if use_bias:
            bqb = pers.tile([P, C], F32)
            bkb = pers.tile([P, C], F32)

        make_identity(nc, ident)
        nc.vector.memset(qn2, 1.0)
        nc.vector.memset(kn2, 1.0)
        nc.sync.dma_start(bq_sb, bq_d[:].rearrange("(c p) -> p c", p=P))
        nc.sync.dma_start(bk_sb, bk_d[:].rearrange("(c p) -> p c", p=P))
        nc.sync.dma_start(bo_sb, bo_d[:].rearrange("(c p) -> p c", p=P))
        if use_bias:
            nc.sync.dma_start(
                bqb, bass.AP(tensor=bq_d[:].tensor, offset=0, ap=[[0, P], [1, C]])
            )
            nc.sync.dma_start(
                bkb, bass.AP(tensor=bk_d[:].tensor, offset=0, ap=[[0, P], [1, C]])
            )

        # ---------- staging pool (released before the attention loop) ----------
        with tc.tile_pool(name="stage", bufs=2) as stage:
            wstg = stage.tile([P, CC, C], F32, tag="wstg", bufs=3)
            nc.sync.dma_start(wstg, wq_d[:].rearrange("(cc p) d -> p cc d", p=P))
            nc.vector.tensor_copy(wq_r, wstg)
            wstg2 = stage.tile([P, CC, C], F32, tag="wstg", bufs=3)
            nc.sync.dma_start(wstg2, wk_d[:].rearrange("(cc p) d -> p cc d", p=P))
            nc.vector.tensor_copy(wk_r, wstg2)
            wstg3 = stage.tile([P, CC, C], F32, tag="wstg", bufs=3)
            nc.sync.dma_start(wstg3, w3_d[:].rearrange("(cc p) d -> p cc d", p=P))
            nc.vector.tensor_copy(w3_r, wstg3)

            ones_f = stage.tile([P, 1], F32, tag="ones")
            nc.vector.memset(ones_f, 1.0)
            # ones column of every pv^T chunk (softmax denominator channel)
            nc.vector.tensor_copy(
                pvT[:, :, C : C + 2], ones_f.broadcast_to([P, len(M_CHUNKS), 2])
            )

            # ---- chunked load+round, interleaved with projections/norms ----
            for n0, nw in N_TILES:
                seg_chunks = [(mi, m0, mw) for mi, (m0, mw) in enumerate(M_CHUNKS)
                              if n0 <= m0 < n0 + nw]
                # --- xq segment: qT norm chunks + q projection ---
                xstg = stage.tile([P, CC, NT], F32, tag="xstg", bufs=4,
                                  name=f"xq_stg{n0}")
                nc.sync.dma_start(xstg[:, :, :nw], xq_v[:, :, n0 : n0 + nw])
                nc.gpsimd.tensor_copy(xq_r[:, :, n0 : n0 + nw], xstg[:, :, :nw])
                for mi, m0, mw in seg_chunks:
                    ps = mm512.tile([P, C], F32, tag="mm512", name=f"qt{m0}")
                    for cc in range(CC):
                        _mm(nc, ps[:mw], xq_r[:, cc, m0 : m0 + mw], wq_r[:, cc, :],
                            cc == 0, cc == CC - 1)
                    scr = small.tile([P, C], F32, tag="sq", bufs=3, name=f"sq{m0}")
                    if use_bias:
                        nc.vector.tensor_add(scr[:mw], ps[:mw], bqb[:mw])
                        nc.scalar.activation(scr[:mw], scr[:mw], AF.Square,
                                             accum_out=qn2[:mw, mi : mi + 1])
                    else:
                        nc.scalar.activation(scr[:mw], ps[:mw], AF.Square,
                                             accum_out=qn2[:mw, mi : mi + 1])
                for dc in range(CC):
                    ps = mm512.tile([P, NT], F32, tag="mm512", name=f"q{n0}_{dc}")
                    for cc in range(CC):
                        _mm(nc, ps[:, :nw], wq_r[:, cc, dc * P : (dc + 1) * P],
                            xq_r[:, cc, n0 : n0 + nw], cc == 0, cc == CC - 1)
                    if use_bias:
                        nc.vector.tensor_scalar_add(q_r[:, dc, n0 : n0 + nw],
                                                    ps[:, :nw], bq_sb[:, dc : dc + 1])
                    else:
                        nc.vector.tensor_copy(q_r[:, dc, n0 : n0 + nw], ps[:, :nw])
                # --- xkv segment: kT norm chunks + k projection + pvT ---
                xstg2 = stage.tile([P, CC, NT], F32, tag="xstg", bufs=4,
                                   name=f"xkv_stg{n0}")
                nc.sync.dma_start(xstg2[:, :, :nw], xkv_v[:, :, n0 : n0 + nw])
                nc.gpsimd.tensor_copy(xkv_r[:, :, n0 : n0 + nw], xstg2[:, :, :nw])
                for mi, m0, mw in seg_chunks:
                    ps = mm512.tile([P, C], F32, tag="mm512", name=f"kt{m0}")
                    for cc in range(CC):
                        _mm(nc, ps[:mw], xkv_r[:, cc, m0 : m0 + mw], wk_r[:, cc, :],
                            cc == 0, cc == CC - 1)
                    scr = small.tile([P, C], F32, tag="sq", bufs=3, name=f"sk{m0}")
                    if use_bias:
                        nc.vector.tensor_add(scr[:mw], ps[:mw], bkb[:mw])
                        nc.scalar.activation(scr[:mw], scr[:mw], AF.Square,
                                             accum_out=kn2[:mw, mi : mi + 1])
                    else:
                        nc.scalar.activation(scr[:mw], ps[:mw], AF.Square,
                                             accum_out=kn2[:mw, mi : mi + 1])
                for dc in range(CC):
                    ps = mm512.tile([P, NT], F32, tag="mm512", name=f"k{n0}_{dc}")
                    for cc in range(CC):
                        _mm(nc, ps[:, :nw], wk_r[:, cc, dc * P : (dc + 1) * P],
                            xkv_r[:, cc, n0 : n0 + nw], cc == 0, cc == CC - 1)
                    if use_bias:
                        nc.vector.tensor_scalar_add(k_r[:, dc, n0 : n0 + nw],
                                                    ps[:, :nw], bk_sb[:, dc : dc + 1])
                    else:
                        nc.vector.tensor_copy(k_r[:, dc, n0 : n0 + nw], ps[:, :nw])
                mi_lo = seg_chunks[0][0]
                mi_hi = seg_chunks[-1][0] + 1
                t0 = stage.tile([P, 4], F32, tag="dn", bufs=3, name=f"dn{n0}")
                nseg = mi_hi - mi_lo
                nc.vector.tensor_mul(t0[:, :nseg], qn2[:, mi_lo:mi_hi],
                                     kn2[:, mi_lo:mi_hi])
                nc.scalar.activation(t0[:, :nseg], t0[:, :nseg], AF.Sqrt)
                nc.vector.tensor_scalar_add(t0[:, :nseg], t0[:, :nseg], EPS)
                nc.vector.reciprocal(rd[:, mi_lo:mi_hi], t0[:, :nseg])
                for mi, m0, mw in seg_chunks:
                    ps = mm512.tile([P, C], F32, tag="mm512", name=f"pv{m0}")
                    for cc in range(CC):
                        _mm(nc, ps[:mw], xkv_r[:, cc, m0 : m0 + mw], w3_r[:, cc, :],
                            cc == 0, cc == CC - 1)
                    nc.vector.tensor_copy(pvT[:mw, mi, :C], ps[:mw])


            # ---- pv^T chunks: (Wproj @ v)^T with m on partitions ----
            for mi, (m0, mw) in enumerate(M_CHUNKS):
                ps = mm512.tile([P, C], F32, tag="mm512")
                for cc in range(CC):
                    _mm(nc, ps[:mw], xkv_r[:, cc, m0 : m0 + mw], w3_r[:, cc, :],
                        cc == 0, cc == CC - 1)
                nc.vector.tensor_copy(pvT[:mw, mi, :C], ps[:mw])

        # ---------- late pools (reuse released staging space) ----------
        e32p = ctx.enter_context(tc.tile_pool(name="e32p", bufs=4))
        erp = ctx.enter_context(tc.tile_pool(name="erp", bufs=4))
        unp = ctx.enter_context(tc.tile_pool(name="unp", bufs=4))
        obp = ctx.enter_context(tc.tile_pool(name="obp", bufs=6))
        rcp = ctx.enter_context(tc.tile_pool(name="rcp", bufs=4))
        tpp = ctx.enter_context(tc.tile_pool(name="tpp", bufs=1, space="PSUM"))

        # ---------- attention main loop ----------
        for n0, nw in N_TILES:
            nsub = (nw + P - 1) // P
            accs = [accp.tile([P, C + 2], F32, tag="acc", name=f"acc{n0}_{s}")
                    for s in range(nsub)]
            n_mc = len(M_CHUNKS)
            for mi, (m0, mw) in enumerate(M_CHUNKS):
                sps = mm512.tile([P, NT], F32, tag="mm512")
                for cc in range(CC):
                    _mm(nc, sps[:mw, :nw], k_r[:, cc, m0 : m0 + mw],
                        q_r[:, cc, n0 : n0 + nw], cc == 0, cc == CC - 1)
                e32 = e32p.tile([P, NT], F32, tag="e32")
                nc.scalar.activation(e32[:mw, :nw], sps[:mw, :nw], AF.Exp,
                                     scale=rd[:mw, mi : mi + 1])
                er = erp.tile([P, NT], F32R, tag="er")
                nc.gpsimd.tensor_copy(er[:mw, :nw], e32[:mw, :nw])
                for s in range(nsub):
                    bw = min(P, nw - s * P)
                    _mm(nc, accs[s][:bw], er[:mw, s * P : s * P + bw],
                        pvT[:mw, mi, :], mi == 0, mi == n_mc - 1)
            for s in range(nsub):
                bw = min(P, nw - s * P)
                rc = rcp.tile([P, 1], F32, tag="rc")
                nc.vector.reciprocal(rc[:bw], accs[s][:bw, C : C + 1])
                un = unp.tile([P, C], F32, tag="un")
                nc.vector.tensor_scalar_mul(un[:bw], accs[s][:bw, :C], rc[:bw])
                for cb in range(CC):
                    tp = tpp.tile([P, P], F32, tag="tp", bufs=1)
                    nc.tensor.transpose(tp[:, :bw], un[:bw, cb * P : (cb + 1) * P],
                                        ident[:bw, :bw])
                    ob = obp.tile([P, P], F32, tag="ob")
                    pos = n0 + s * P
                    # + residual (x_q) and output bias
                    nc.vector.tensor_add(ob[:, :bw], tp[:, :bw],
                                         xq_r[:, cb, pos : pos + bw])
                    if use_bias:
                        nc.vector.tensor_scalar_add(ob[:, :bw], ob[:, :bw],
                                                    bo_sb[:, cb : cb + 1])
                    nc.sync.dma_start(out_v[:, cb, pos : pos + bw], ob[:, :bw])

    return nc


_CACHE = {}


def _get_module(use_bias: bool):
    key = use_bias
    if key not in _CACHE:
        nc = build(use_bias)
        nc.finalize()
        _CACHE[key] = nc
    return _CACHE[key]


def kernel(x_q, x_kv, Wq, bq, Wkv, bkv, Wproj, bproj):
    x_q = np.asarray(x_q, dtype=np.float32)
    x_kv = np.asarray(x_kv, dtype=np.float32)
    Wq = np.asarray(Wq, dtype=np.float32)
    bq = np.asarray(bq, dtype=np.float32)
    Wkv = np.asarray(Wkv, dtype=np.float32)
    bkv = np.asarray(bkv, dtype=np.float32)
    Wproj = np.asarray(Wproj, dtype=np.float32)
    bproj = np.asarray(bproj, dtype=np.float32)

    B, c, H, W = x_q.shape
    assert (c, H * W) == (C, N), (x_q.shape,)
    xq = np.ascontiguousarray(x_q.reshape(B, C, N))
    xkv = np.ascontiguousarray(x_kv.reshape(B, C, N))

    Wk = Wkv[:C]
    Wv = Wkv[C:]
    wqT = np.ascontiguousarray(Wq.T)
    wkT = np.ascontiguousarray(Wk.T)
    w3T = np.ascontiguousarray((Wproj @ Wv).T)
    bk = np.ascontiguousarray(bkv[:C])
    bo = np.ascontiguousarray(Wproj @ bkv[C:] + bproj)

    use_bias = bool(np.any(bq) or np.any(bk) or np.any(bo))
    nc = _get_module(use_bias)

    in_maps = [
        {
            "xq": xq[b],
            "xkv": xkv[b],
            "wqT": wqT,
            "wkT": wkT,
            "w3T": w3T,
            "bq": bq,
            "bk": bk,
            "bo": bo,
        }
        for b in range(B)
    ]
    res = run_bass_kernel_spmd(nc, in_maps, core_ids=list(range(B)))
    out = np.stack([res.results[b]["out"] for b in range(B)], axis=0)
    return out.reshape(B, C, H, W)



# revision 2
# speedup vs baseline: 1.0936x; 1.0936x over previous
"""CrossAttention Trainium2 kernel, fp8 DoubleRow edition (1 batch elem/core).

Math per batch element (C=256 channels, N=3136 positions):
    q = Wq xq, k = Wk xkv, pv = (Wproj Wv) xkv          (1x1 convs)
    S[n,m] = q[:,n].k[:,m];  rd[m] = 1/(||q[:,m]|| ||k[:,m]|| + eps)
    A = softmax(S * rd[m] along m);  out = Wproj(A v) + xq residual

Implementation notes:
  * All matmuls in fp8e4 DoubleRow mode (2 fp8 rows/cycle, K=256 in one op).
    Weights and x are scaled by 16 on the host so w entries ~N(0,1) stay out
    of fp8 subnormals. Scales self-cancel in softmax: q,k scale 16 each =>
    S x256, qn2*kn2 x65536, rd = rsqrt(qn2*kn2) absorbs it. pv scale 16 is
    divided out in the output STT.
  * S^T tiles [m=128, n<=1024] (2 psum banks) -> ONE ACT Exp per (chunk,
    n-super) with per-partition scale rd[:,mi], writing fp8 er directly.
    |S*rd| <= ~1.1 so no max-subtraction is needed (exp in [0.3, 3.1]).
  * Norms via transposed DR projections (qT [m,256] psum) + DVE square +
    X-reduce; rd = rsqrt via DVE bit-trick + 2 Newton steps (ACT Sqrt would
    thrash the Exp activation table; gpsimd cannot read PSUM).
  * AV: er (stationary) x pvT|1,1 (moving) accumulating [n-sub, C+2]; the
    ones channels give the softmax row-sum for free. Normalize on DVE,
    bf16 PE transpose back to [c, n], fused (tp/16 + xq) STT, DMA out.
  * PSUM: S pool 2x2 banks + aux pool 4x1 bank (phase-1 qT/kT/pv/proj tiles,
    then AV acc + transpose) = 8 banks exactly.
"""

import sys

if "/opt/trn_rl_repo" not in sys.path:
    sys.path.insert(0, "/opt/trn_rl_repo")

import numpy as np
import ml_dtypes

import concourse.bass as bass
import concourse.mybir as mybir
import concourse.tile as tile
from concourse import bacc
from concourse.bass_utils import run_bass_kernel_spmd
from concourse.masks import make_identity
from contextlib import ExitStack

F32 = mybir.dt.float32
F8 = mybir.dt.float8e4
BF16 = mybir.dt.bfloat16
I32 = mybir.dt.int32
AF = mybir.ActivationFunctionType
DR = mybir.MatmulPerfMode.DoubleRow
ALU = mybir.AluOpType

P = 128
C = 256
CC = C // P            # 2
N = 56 * 56            # 3136
SEG = 512
SEGS = [(i, min(SEG, N - i)) for i in range(0, N, SEG)]            # 7
M_CHUNKS = [(i, min(P, N - i)) for i in range(0, N, P)]            # 25
N_FULL = len(M_CHUNKS) - 1                                         # 24 full
SUPERS = [(0, 1024), (1024, 1024), (2048, 1024), (3072, 64)]
WSCALE = 16.0          # host-side weight/x scale for fp8 range
# pv is needed only once AV starts (after the last seg): keep its psum
# traffic and DVE casts out of the phase-1 production pipeline entirely
PV_PLAN = {6: range(0, 25)}


def _chunks_of_seg(si):
    lo = si * 4
    return [(mi, M_CHUNKS[mi][0], M_CHUNKS[mi][1])
            for mi in range(lo, min(lo + 4, len(M_CHUNKS)))]


def build(use_bias: bool):
    nc = bacc.Bacc(None, target_bir_lowering=False)

    xq_d = nc.dram_tensor("xq", [C, N], F32, kind="ExternalInput")
    xkv8_d = nc.dram_tensor("xkv8", [C, N], F8, kind="ExternalInput")
    wq8_d = nc.dram_tensor("wq8", [C, C], F8, kind="ExternalInput")   # [c, d]
    wk8_d = nc.dram_tensor("wk8", [C, C], F8, kind="ExternalInput")
    w38_d = nc.dram_tensor("w38", [C, C], F8, kind="ExternalInput")
    bq_d = nc.dram_tensor("bq16", [C], F32, kind="ExternalInput")     # 16*bq
    bk_d = nc.dram_tensor("bk16", [C], F32, kind="ExternalInput")     # 16*bk
    bo_d = nc.dram_tensor("bo", [C], F32, kind="ExternalInput")       # Wproj bv + bproj
    out_d = nc.dram_tensor("out", [C, N], F32, kind="ExternalOutput")

    xq_v = xq_d[:].rearrange("(cc p) n -> p cc n", p=P)
    xkv8_v = xkv8_d[:].rearrange("(cc p) n -> p cc n", p=P)
    out_v = out_d[:].rearrange("(cc p) n -> p cc n", p=P)

    with tile.TileContext(nc) as tc, ExitStack() as ctx:
        pers = ctx.enter_context(tc.tile_pool(name="pers", bufs=1))
        sS = ctx.enter_context(tc.tile_pool(name="sS", bufs=2, space="PSUM"))
        aux = ctx.enter_context(tc.tile_pool(name="aux", bufs=4, space="PSUM"))
        scrp = ctx.enter_context(tc.tile_pool(name="scrp", bufs=3))
        e8p = ctx.enter_context(tc.tile_pool(name="e8p", bufs=39))
        e8s = ctx.enter_context(tc.tile_pool(name="e8s", bufs=13))
        outp = ctx.enter_context(tc.tile_pool(name="outp", bufs=4))
        smls = ctx.enter_context(tc.tile_pool(name="smls", bufs=4))

        xq_f = pers.tile([P, CC, N], F32)
        x8q = pers.tile([P, CC, N], F8)
        x8kv = pers.tile([P, CC, N], F8)
        q8 = pers.tile([P, CC, N], F8)
        k8 = pers.tile([P, CC, N], F8)
        wq8 = pers.tile([P, CC, C], F8)
        wk8 = pers.tile([P, CC, C], F8)
        w38 = pers.tile([P, CC, C], F8)
        pvT8 = pers.tile([P, 13, 2, C + 2], F8)
        qn2 = pers.tile([P, len(M_CHUNKS)], F32)
        kn2 = pers.tile([P, len(M_CHUNKS)], F32)
        rd = pers.tile([P, len(M_CHUNKS)], F32)
        ident = pers.tile([P, P], BF16)
        if use_bias:
            bq_sb = pers.tile([P, CC], F32)
            bk_sb = pers.tile([P, CC], F32)
            bo_sb = pers.tile([P, CC], F32)
            bqb = pers.tile([P, C], F32)
            bkb = pers.tile([P, C], F32)

        def preamble():
            make_identity(nc, ident)
            nc.sync.dma_start(wq8, wq8_d[:].rearrange("(cc p) d -> p cc d", p=P))
            nc.sync.dma_start(wk8, wk8_d[:].rearrange("(cc p) d -> p cc d", p=P))
            nc.sync.dma_start(w38, w38_d[:].rearrange("(cc p) d -> p cc d", p=P))
            # softmax row-sum channels
            nc.vector.memset(pvT8[:, :, :, C : C + 2], 1.0)
        if use_bias:
            nc.sync.dma_start(bq_sb, bq_d[:].rearrange("(c p) -> p c", p=P))
            nc.sync.dma_start(bk_sb, bk_d[:].rearrange("(c p) -> p c", p=P))
            nc.sync.dma_start(bo_sb, bo_d[:].rearrange("(c p) -> p c", p=P))
            nc.sync.dma_start(
                bqb, bass.AP(tensor=bq_d[:].tensor, offset=0, ap=[[0, P], [1, C]])
            )
            nc.sync.dma_start(
                bkb, bass.AP(tensor=bk_d[:].tensor, offset=0, ap=[[0, P], [1, C]])
            )

        # ---------------- phase 1 (per 512-seg, pipelined) ----------------
        def norms_for_chunk(mi, m0, mw, on_act=False):
            for which, xsrc, w8, nacc, bb in (
                ("q", x8q, wq8, qn2, "bqb"),
                ("k", x8kv, wk8, kn2, "bkb"),
            ):
                ps = aux.tile([P, SEG], F32, tag="aux", name=f"t{which}{m0}")
                nc.tensor.matmul(ps[:mw, :C], xsrc[:, :, m0 : m0 + mw], w8,
                                 start=True, stop=True, perf_mode=DR)
                scr = scrp.tile([P, C], F32, tag="sq", name=f"s{which}{m0}")
                if use_bias:
                    bbt = bqb if which == "q" else bkb
                    nc.vector.tensor_add(scr[:mw], ps[:mw, :C], bbt[:mw])
                    nc.vector.scalar_tensor_tensor(
                        scr[:mw], scr[:mw], 1.0, scr[:mw], ALU.mult, ALU.mult,
                        accum_out=nacc[:mw, mi : mi + 1])
                else:
                    # hw allows only one PSUM operand per DVE op: stage the
                    # qT tile to SBUF (bf16 for 2x DVE mode), then square+
                    # accumulate in a second DVE op
                    scrh = scrp.tile([P, C], BF16, tag="sqh",
                                     name=f"h{which}{m0}")
                    nc.vector.tensor_copy(scrh[:mw], ps[:mw, :C])
                    sqo = scrp.tile([P, C], BF16, tag="sqo",
                                    name=f"o{which}{m0}")
                    nc.vector.scalar_tensor_tensor(
                        sqo[:mw], scrh[:mw], 1.0, scrh[:mw], ALU.mult,
                        ALU.mult, accum_out=nacc[:mw, mi : mi + 1])

        def pv_for_chunk(mi, m0, mw):
            ps = aux.tile([P, SEG], F32, tag="aux", name=f"pv{m0}")
            nc.tensor.matmul(ps[:mw, :C], x8kv[:, :, m0 : m0 + mw], w38,
                             start=True, stop=True, perf_mode=DR)
            nc.vector.tensor_copy(pvT8[:mw, mi // 2, mi % 2, :C], ps[:mw, :C])

        def rd_for_seg(si):
            lo = si * 4
            hi = min(lo + 4, len(M_CHUNKS))
            nseg = hi - lo
            u = smls.tile([P, 4], F32, tag="u", name=f"u{si}")
            nc.vector.tensor_mul(u[:, :nseg], qn2[:, lo:hi], kn2[:, lo:hi])
            yb = smls.tile([P, 4], I32, tag="yb", name=f"yb{si}")
            nc.vector.tensor_scalar(yb[:, :nseg], u[:, :nseg].bitcast(I32),
                                    1, None, ALU.logical_shift_right)
            nc.vector.tensor_scalar(yb[:, :nseg], yb[:, :nseg], -1, 0x5F3759DF,
                                    ALU.mult, ALU.add)
            y = yb.bitcast(F32)
            h = smls.tile([P, 4], F32, tag="h", name=f"h{si}")
            for _ in range(2):
                nc.vector.tensor_mul(h[:, :nseg], y[:, :nseg], y[:, :nseg])
                nc.vector.tensor_mul(h[:, :nseg], h[:, :nseg], u[:, :nseg])
                nc.vector.tensor_scalar(h[:, :nseg], h[:, :nseg], -0.5, 1.5,
                                        ALU.mult, ALU.add)
                nc.vector.tensor_mul(y[:, :nseg], y[:, :nseg], h[:, :nseg])
            nc.vector.tensor_copy(rd[:, lo:hi], y[:, :nseg])

        def proj_for_seg(n0, nw, on_act=False):
            for which, xsrc, w8, dst, bsb in (
                ("q", x8q, wq8, q8, "bq_sb"),
                ("k", x8kv, wk8, k8, "bk_sb"),
            ):
                for dc in range(CC):
                    ps = aux.tile([P, SEG], F32, tag="aux",
                                  name=f"p{which}{n0}_{dc}")
                    nc.tensor.matmul(ps[:, :nw], w8[:, :, dc * P : (dc + 1) * P],
                                     xsrc[:, :, n0 : n0 + nw],
                                     start=True, stop=True, perf_mode=DR)
                    if use_bias:
                        bt = bq_sb if which == "q" else bk_sb
                        nc.vector.tensor_scalar(dst[:, dc, n0 : n0 + nw],
                                                ps[:, :nw], bt[:, dc : dc + 1],
                                                None, ALU.add)
                    elif on_act:
                        # lead-in: ACT is otherwise idle, Copy shares the
                        # Exp table, and this shortens the DVE dep chain
                        # in front of the first exps
                        nc.scalar.activation(dst[:, dc, n0 : n0 + nw],
                                             ps[:, :nw], AF.Copy)
                    else:
                        nc.vector.tensor_copy(dst[:, dc, n0 : n0 + nw],
                                              ps[:, :nw])

        def dma_seg(si):
            n0, nw = SEGS[si]
            nc.sync.dma_start(xq_f[:, :, n0 : n0 + nw], xq_v[:, :, n0 : n0 + nw])
            nc.sync.dma_start(x8kv[:, :, n0 : n0 + nw], xkv8_v[:, :, n0 : n0 + nw])
            nc.gpsimd.tensor_copy(x8q[:, :, n0 : n0 + nw],
                                  xq_f[:, :, n0 : n0 + nw])

        def compute_seg(si):
            n0, nw = SEGS[si]
            for mi, m0, mw in _chunks_of_seg(si):
                norms_for_chunk(mi, m0, mw)
            rd_for_seg(si)
            proj_for_seg(n0, nw, on_act=(si <= 1))

        # ---------------- phase 2 ----------------
        er_tiles = {}

        def s_exp_chunk(sj, mi):
            sn0, snw = SUPERS[sj]
            m0, mw = M_CHUNKS[mi]
            sp = sS.tile([P, 2, SEG], F32, tag="sp", name=f"sp{sj}_{mi}")
            halves = [(0, 512), (512, 512)] if snw == 1024 else [(0, snw)]
            for hi, (h0, hw) in enumerate(halves):
                nc.tensor.matmul(sp[:mw, hi, :hw], k8[:, :, m0 : m0 + mw],
                                 q8[:, :, sn0 + h0 : sn0 + h0 + hw],
                                 start=True, stop=True, perf_mode=DR)
            pi, slot = mi // 2, mi % 2
            key = (sj, pi)
            if key not in er_tiles:
                if snw > SEG:
                    er_tiles[key] = e8p.tile([P, 2, 2, SEG], F8, tag="er",
                                             name=f"er{sj}_{pi}")
                else:
                    er_tiles[key] = e8s.tile([P, 2, 1, SEG], F8, tag="ers",
                                             name=f"er{sj}_{pi}")
            er = er_tiles[key]
            if snw == 1024:
                nc.scalar.activation(er[:mw, slot, :, :], sp[:mw, :, :], AF.Exp,
                                     scale=rd[:mw, mi : mi + 1])
            else:
                nc.scalar.activation(er[:mw, slot, 0, :snw], sp[:mw, 0, :snw],
                                     AF.Exp, scale=rd[:mw, mi : mi + 1])

        def av_out_sub(sj, s):
                sn0, snw = SUPERS[sj]
                bw = min(P, snw - s * P)
                hh, c0 = s // 4, (s % 4) * P
                acc = aux.tile([P, SEG], F32, tag="aux", name=f"acc{sj}_{s}")
                for pi in range(12):
                    er = er_tiles[(sj, pi)]
                    nc.tensor.matmul(acc[:bw, : C + 2],
                                     er[:, :, hh, c0 : c0 + bw],
                                     pvT8[:, pi, :, :],
                                     start=(pi == 0), stop=False, perf_mode=DR)
                er = er_tiles[(sj, 12)]
                lmw = M_CHUNKS[24][1]
                nc.tensor.matmul(acc[:bw, : C + 2],
                                 er[:lmw, 0, hh, c0 : c0 + bw],
                                 pvT8[:lmw, 12, 0, :],
                                 start=False, stop=True)
                # normalize: un = acc[:, :C] * (1/rowsum), bf16
                rc = smls.tile([P, 1], F32, tag="rc", name=f"rc{sj}_{s}")
                nc.vector.reciprocal(rc[:bw], acc[:bw, C : C + 1])
                un = scrp.tile([P, C], BF16, tag="un", name=f"un{sj}_{s}")
                nc.vector.tensor_scalar(un[:bw], acc[:bw, :C], rc[:bw], None,
                                        ALU.mult)
                pos = sn0 + s * P
                for cb in range(CC):
                    tp = aux.tile([P, 2 * SEG], BF16, tag="aux",
                                  name=f"tp{sj}_{s}_{cb}")
                    nc.tensor.transpose(tp[:, :bw], un[:bw, cb * P : (cb + 1) * P],
                                        ident[:bw, :bw])
                    ot = outp.tile([P, CC, P], F32, tag="ot",
                                   name=f"ot{sj}_{s}") if cb == 0 else ot
                    nc.vector.scalar_tensor_tensor(
                        ot[:, cb, :bw], tp[:, :bw], 1.0 / WSCALE,
                        xq_f[:, cb, pos : pos + bw], ALU.mult, ALU.add)
                    if use_bias:
                        nc.vector.tensor_scalar(ot[:, cb, :bw], ot[:, cb, :bw],
                                                bo_sb[:, cb : cb + 1], None,
                                                ALU.add)
                nc.sync.dma_start(out_v[:, :, pos : pos + bw], ot[:, :, :bw])

        def av_out_super(sj):
            snw = SUPERS[sj][1]
            for s in range((snw + P - 1) // P):
                av_out_sub(sj, s)

        # Work-queue emission: an exp for (super sj, chunk mi) is ready once
        # the q8 segs covering the super and the k8/rd seg covering the chunk
        # are computed. Emitting in availability order keeps the ACT queue
        # full from ~seg 2 onward. AV/output subtiles of completed supers are
        # interleaved between exps so the PE queue always has ready work.
        sup_ready_at = [(sn0 + snw - 1) // SEG for sn0, snw in SUPERS]
        n_chunks = len(M_CHUNKS)
        ptr = [0] * len(SUPERS)   # next chunk to emit per super
        av_pending = []
        av_done = 0
        FILL = 3

        def emit_av(k):
            nonlocal av_done
            while av_done < k and av_done < len(av_pending):
                av_out_sub(*av_pending[av_done])
                av_done += 1

        def emit_exp(sj, mi, av_rate=1):
            s_exp_chunk(sj, mi)
            ptr[sj] = mi + 1
            if mi == n_chunks - 1:
                nsub = (SUPERS[sj][1] + P - 1) // P
                av_pending.extend((sj, s) for s in range(nsub))
            emit_av(av_done + av_rate)

        dma_seg(0)
        preamble()
        for si in range(len(SEGS)):
            if si + 1 < len(SEGS):
                dma_seg(si + 1)
            compute_seg(si)
            for mi2 in PV_PLAN.get(si, ()):
                pv_for_chunk(mi2, *M_CHUNKS[mi2])
            avail = min(4 * (si + 1), n_chunks)
            # super 0 has priority: its AV (and every later super's) can only
            # drain after the last seg, so finish earlier supers first and
            # fill ACT with just a little of the next super to avoid gaps.
            if sup_ready_at[0] <= si:
                for mi in range(ptr[0], avail):
                    emit_exp(0, mi)
            if si >= 1:
                for sj in range(1, len(SUPERS)):
                    if sup_ready_at[sj] > si or ptr[sj] >= avail:
                        continue
                    for mi in range(ptr[sj], min(ptr[sj] + FILL, avail)):
                        emit_exp(sj, mi)
                    break
        # drain remaining supers, AV interleaved; the tiny last super is
        # drained second-to-last so the final super's exps cover its AV
        for sj in range(1, len(SUPERS)):
            for mi in range(ptr[sj], n_chunks):
                emit_exp(sj, mi, av_rate=2)
        emit_av(len(av_pending))

    return nc


_CACHE = {}


def _get_module(use_bias: bool):
    if use_bias not in _CACHE:
        nc = build(use_bias)
        nc.finalize()
        _CACHE[use_bias] = nc
    return _CACHE[use_bias]


def kernel(x_q, x_kv, Wq, bq, Wkv, bkv, Wproj, bproj):
    x_q = np.asarray(x_q, dtype=np.float32)
    x_kv = np.asarray(x_kv, dtype=np.float32)
    Wq = np.asarray(Wq, dtype=np.float32)
    bq = np.asarray(bq, dtype=np.float32)
    Wkv = np.asarray(Wkv, dtype=np.float32)
    bkv = np.asarray(bkv, dtype=np.float32)
    Wproj = np.asarray(Wproj, dtype=np.float32)
    bproj = np.asarray(bproj, dtype=np.float32)

    B, c, H, W = x_q.shape
    assert (c, H * W) == (C, N), (x_q.shape,)
    FP8 = ml_dtypes.float8_e4m3
    xq = np.ascontiguousarray(x_q.reshape(B, C, N))
    xkv8 = np.ascontiguousarray(x_kv.reshape(B, C, N)).astype(FP8)

    Wk = Wkv[:C]
    Wv = Wkv[C:]
    wq8 = np.ascontiguousarray(WSCALE * Wq.T).astype(FP8)
    wk8 = np.ascontiguousarray(WSCALE * Wk.T).astype(FP8)
    w38 = np.ascontiguousarray(WSCALE * (Wproj @ Wv).T).astype(FP8)
    bq16 = np.ascontiguousarray(WSCALE * bq)
    bk16 = np.ascontiguousarray(WSCALE * bkv[:C])
    bo = np.ascontiguousarray(Wproj @ bkv[C:] + bproj)

    use_bias = bool(np.any(bq16) or np.any(bk16) or np.any(bo))
    nc = _get_module(use_bias)

    in_maps = [
        {
            "xq": xq[b],
            "xkv8": xkv8[b],
            "wq8": wq8,
            "wk8": wk8,
            "w38": w38,
            "bq16": bq16,
            "bk16": bk16,
            "bo": bo,
        }
        for b in range(B)
    ]
    res = run_bass_kernel_spmd(nc, in_maps, core_ids=list(range(B)))
    out = np.stack([res.results[b]["out"] for b in range(B)], axis=0)
    return out.reshape(B, C, H, W)


# revision 3
# speedup vs baseline: 1.1117x; 1.0165x over previous
"""CrossAttention Trainium2 kernel, fp8 DoubleRow edition (1 batch elem/core).

Math per batch element (C=256 channels, N=3136 positions):
    q = Wq xq, k = Wk xkv, pv = (Wproj Wv) xkv          (1x1 convs)
    S[n,m] = q[:,n].k[:,m];  rd[m] = 1/(||q[:,m]|| ||k[:,m]|| + eps)
    A = softmax(S * rd[m] along m);  out = Wproj(A v) + xq residual

Implementation notes:
  * All matmuls in fp8e4 DoubleRow mode (2 fp8 rows/cycle, K=256 in one op).
    Weights and x are scaled by 16 on the host so w entries ~N(0,1) stay out
    of fp8 subnormals. Scales self-cancel in softmax: q,k scale 16 each =>
    S x256, qn2*kn2 x65536, rd = rsqrt(qn2*kn2) absorbs it. pv scale 16 is
    divided out in the output STT.
  * S^T tiles [m=128, n<=1024] (2 psum banks) -> ONE ACT Exp per (chunk,
    n-super) with per-partition scale rd[:,mi], writing fp8 er directly.
    |S*rd| <= ~1.1 so no max-subtraction is needed (exp in [0.3, 3.1]).
  * Norms via transposed DR projections (qT [m,256] psum) + DVE square +
    X-reduce; rd = rsqrt via DVE bit-trick + 2 Newton steps (ACT Sqrt would
    thrash the Exp activation table; gpsimd cannot read PSUM).
  * AV: er (stationary) x pvT|1,1 (moving) accumulating [n-sub, C+2]; the
    ones channels give the softmax row-sum for free. Normalize on DVE,
    bf16 PE transpose back to [c, n], fused (tp/16 + xq) STT, DMA out.
  * PSUM: S pool 2x2 banks + aux pool 4x1 bank (phase-1 qT/kT/pv/proj tiles,
    then AV acc + transpose) = 8 banks exactly.
"""

import sys

if "/opt/trn_rl_repo" not in sys.path:
    sys.path.insert(0, "/opt/trn_rl_repo")

import numpy as np
import ml_dtypes

import concourse.bass as bass
import concourse.mybir as mybir
import concourse.tile as tile
from concourse import bacc
from concourse.bass_utils import run_bass_kernel_spmd
from concourse.masks import make_identity
from contextlib import ExitStack

F32 = mybir.dt.float32
F8 = mybir.dt.float8e4
BF16 = mybir.dt.bfloat16
I32 = mybir.dt.int32
AF = mybir.ActivationFunctionType
DR = mybir.MatmulPerfMode.DoubleRow
ALU = mybir.AluOpType

P = 128
C = 256
CC = C // P            # 2
N = 56 * 56            # 3136
SEG = 512
SEGS = [(i, min(SEG, N - i)) for i in range(0, N, SEG)]            # 7
M_CHUNKS = [(i, min(P, N - i)) for i in range(0, N, P)]            # 25
N_FULL = len(M_CHUNKS) - 1                                         # 24 full
SUPERS = [(0, 1024), (1024, 1024), (2048, 1024), (3072, 64)]
WSCALE = 16.0          # host-side weight/x scale for fp8 range
# pv is needed only once AV starts (after the last seg): keep its psum
# traffic and DVE casts out of the phase-1 production pipeline entirely
PV_PLAN = {6: range(0, 25)}


def _chunks_of_seg(si):
    lo = si * 4
    return [(mi, M_CHUNKS[mi][0], M_CHUNKS[mi][1])
            for mi in range(lo, min(lo + 4, len(M_CHUNKS)))]


def build(use_bias: bool):
    nc = bacc.Bacc(None, target_bir_lowering=False)

    xq_d = nc.dram_tensor("xq", [C, N], F32, kind="ExternalInput")
    xkv8_d = nc.dram_tensor("xkv8", [C, N], F8, kind="ExternalInput")
    wq8_d = nc.dram_tensor("wq8", [C, C], F8, kind="ExternalInput")   # [c, d]
    wk8_d = nc.dram_tensor("wk8", [C, C], F8, kind="ExternalInput")
    w38_d = nc.dram_tensor("w38", [C, C], F8, kind="ExternalInput")
    gw8_d = nc.dram_tensor("gw8", [C, C], F8, kind="ExternalInput")   # 16*Wq.T@Wk
    bq_d = nc.dram_tensor("bq16", [C], F32, kind="ExternalInput")     # 16*bq
    bk_d = nc.dram_tensor("bk16", [C], F32, kind="ExternalInput")     # 16*bk
    bo_d = nc.dram_tensor("bo", [C], F32, kind="ExternalInput")       # Wproj bv + bproj
    out_d = nc.dram_tensor("out", [C, N], F32, kind="ExternalOutput")

    xq_v = xq_d[:].rearrange("(cc p) n -> p cc n", p=P)
    xkv8_v = xkv8_d[:].rearrange("(cc p) n -> p cc n", p=P)
    out_v = out_d[:].rearrange("(cc p) n -> p cc n", p=P)

    with tile.TileContext(nc) as tc, ExitStack() as ctx:
        pers = ctx.enter_context(tc.tile_pool(name="pers", bufs=1))
        sS = ctx.enter_context(tc.tile_pool(name="sS", bufs=2, space="PSUM"))
        aux = ctx.enter_context(tc.tile_pool(name="aux", bufs=4, space="PSUM"))
        scrp = ctx.enter_context(tc.tile_pool(name="scrp", bufs=3))
        e8p = ctx.enter_context(tc.tile_pool(name="e8p", bufs=39))
        e8s = ctx.enter_context(tc.tile_pool(name="e8s", bufs=13))
        outp = ctx.enter_context(tc.tile_pool(name="outp", bufs=4))
        smls = ctx.enter_context(tc.tile_pool(name="smls", bufs=4))

        xq_f = pers.tile([P, CC, N], F32)
        x8q = pers.tile([P, CC, N], F8)
        x8kv = pers.tile([P, CC, N], F8)
        if use_bias:
            q8 = pers.tile([P, CC, N], F8)
            k8 = pers.tile([P, CC, N], F8)
        else:
            # S = xkv^T (Wk^T Wq) xq: one fused projection g = G xq replaces
            # both q and k projections for the S matmul (norms still use
            # the transposed qT/kT products)
            g8 = pers.tile([P, CC, N], F8)
            gw8 = pers.tile([P, CC, C], F8)
        wq8 = pers.tile([P, CC, C], F8)
        wk8 = pers.tile([P, CC, C], F8)
        w38 = pers.tile([P, CC, C], F8)
        pvT8 = pers.tile([P, 13, 2, C + 2], F8)
        rd = pers.tile([P, len(M_CHUNKS)], F32)
        if use_bias:
            qn2 = pers.tile([P, len(M_CHUNKS)], F32)
            kn2 = pers.tile([P, len(M_CHUNKS)], F32)
        ident = pers.tile([P, P], BF16)
        if use_bias:
            bq_sb = pers.tile([P, CC], F32)
            bk_sb = pers.tile([P, CC], F32)
            bo_sb = pers.tile([P, CC], F32)
            bqb = pers.tile([P, C], F32)
            bkb = pers.tile([P, C], F32)

        def preamble():
            make_identity(nc, ident)
            nc.sync.dma_start(wq8, wq8_d[:].rearrange("(cc p) d -> p cc d", p=P))
            nc.sync.dma_start(wk8, wk8_d[:].rearrange("(cc p) d -> p cc d", p=P))
            nc.sync.dma_start(w38, w38_d[:].rearrange("(cc p) d -> p cc d", p=P))
            if not use_bias:
                nc.sync.dma_start(gw8,
                                  gw8_d[:].rearrange("(cc p) d -> p cc d", p=P))
            # softmax row-sum channels
            nc.vector.memset(pvT8[:, :, :, C : C + 2], 1.0)
        if use_bias:
            nc.sync.dma_start(bq_sb, bq_d[:].rearrange("(c p) -> p c", p=P))
            nc.sync.dma_start(bk_sb, bk_d[:].rearrange("(c p) -> p c", p=P))
            nc.sync.dma_start(bo_sb, bo_d[:].rearrange("(c p) -> p c", p=P))
            nc.sync.dma_start(
                bqb, bass.AP(tensor=bq_d[:].tensor, offset=0, ap=[[0, P], [1, C]])
            )
            nc.sync.dma_start(
                bkb, bass.AP(tensor=bk_d[:].tensor, offset=0, ap=[[0, P], [1, C]])
            )

        # ---------------- phase 1 (per 512-seg, pipelined) ----------------
        def norms_for_chunk(mi, m0, mw, ci, bag):
            for side, (which, xsrc, w8) in enumerate(
                (("q", x8q, wq8), ("k", x8kv, wk8))
            ):
                ps = aux.tile([P, SEG], F32, tag="aux", name=f"t{which}{m0}")
                nc.tensor.matmul(ps[:mw, :C], xsrc[:, :, m0 : m0 + mw], w8,
                                 start=True, stop=True, perf_mode=DR)
                if use_bias:
                    nacc = qn2 if which == "q" else kn2
                    bbt = bqb if which == "q" else bkb
                    scr = scrp.tile([P, C], F32, tag="sq", name=f"s{which}{m0}")
                    nc.vector.tensor_add(scr[:mw], ps[:mw, :C], bbt[:mw])
                    nc.vector.scalar_tensor_tensor(
                        scr[:mw], scr[:mw], 1.0, scr[:mw], ALU.mult, ALU.mult,
                        accum_out=nacc[:mw, mi : mi + 1])
                else:
                    # sum(x^2) = n*(var + mean^2) via bn_stats: one DVE pass
                    # over the PSUM tile instead of copy+square+reduce
                    bn6 = scrp.tile([P, 2, 6], F32, tag="bn6",
                                    name=f"b{which}{m0}")
                    nc.vector.bn_stats(bn6[:mw, side, :], ps[:mw, :C])
                    nc.vector.bn_aggr(bag[:mw, ci, side, :], bn6[:mw, side, :])

        def pv_for_chunk(mi, m0, mw):
            ps = aux.tile([P, SEG], F32, tag="aux", name=f"pv{m0}")
            nc.tensor.matmul(ps[:mw, :C], x8kv[:, :, m0 : m0 + mw], w38,
                             start=True, stop=True, perf_mode=DR)
            nc.vector.tensor_copy(pvT8[:mw, mi // 2, mi % 2, :C], ps[:mw, :C])

        def rd_for_seg(si, bag):
            lo = si * 4
            hi = min(lo + 4, len(M_CHUNKS))
            nseg = hi - lo
            u = smls.tile([P, 4], F32, tag="u", name=f"u{si}")
            if use_bias:
                nc.vector.tensor_mul(u[:, :nseg], qn2[:, lo:hi], kn2[:, lo:hi])
            else:
                t = smls.tile([P, 4, 2, 1], F32, tag="tvm", name=f"tvm{si}")
                mean = bag[:, :nseg, :, 0:1]
                var = bag[:, :nseg, :, 1:2]
                nc.vector.scalar_tensor_tensor(t[:, :nseg], mean, 1.0, mean,
                                               ALU.mult, ALU.mult)
                nc.vector.tensor_add(t[:, :nseg], t[:, :nseg], var)
                nc.vector.tensor_mul(u[:, :nseg], t[:, :nseg, 0, 0],
                                     t[:, :nseg, 1, 0])
            yb = smls.tile([P, 4], I32, tag="yb", name=f"yb{si}")
            nc.vector.tensor_scalar(yb[:, :nseg], u[:, :nseg].bitcast(I32),
                                    1, None, ALU.logical_shift_right)
            nc.vector.tensor_scalar(yb[:, :nseg], yb[:, :nseg], -1, 0x5F3759DF,
                                    ALU.mult, ALU.add)
            y = yb.bitcast(F32)
            h = smls.tile([P, 4], F32, tag="h", name=f"h{si}")
            for _ in range(2):
                nc.vector.tensor_mul(h[:, :nseg], y[:, :nseg], y[:, :nseg])
                nc.vector.tensor_mul(h[:, :nseg], h[:, :nseg], u[:, :nseg])
                nc.vector.tensor_scalar(h[:, :nseg], h[:, :nseg], -0.5, 1.5,
                                        ALU.mult, ALU.add)
                nc.vector.tensor_mul(y[:, :nseg], y[:, :nseg], h[:, :nseg])
            if use_bias:
                nc.vector.tensor_copy(rd[:, lo:hi], y[:, :nseg])
            else:
                # g-fold S is 16*S; u = (qn*kn/256)^2 => rd = rsqrt(u)/16
                nc.vector.tensor_scalar(rd[:, lo:hi], y[:, :nseg], 1.0 / 16.0,
                                        None, ALU.mult)

        def proj_for_seg(n0, nw, on_act=False):
            if use_bias:
                plan = (("q", x8q, wq8, q8, bq_sb), ("k", x8kv, wk8, k8, bk_sb))
            else:
                plan = (("g", x8q, gw8, g8, None),)
            for which, xsrc, w8, dst, bt in plan:
                for dc in range(CC):
                    ps = aux.tile([P, SEG], F32, tag="aux",
                                  name=f"p{which}{n0}_{dc}")
                    nc.tensor.matmul(ps[:, :nw], w8[:, :, dc * P : (dc + 1) * P],
                                     xsrc[:, :, n0 : n0 + nw],
                                     start=True, stop=True, perf_mode=DR)
                    if use_bias:
                        nc.vector.tensor_scalar(dst[:, dc, n0 : n0 + nw],
                                                ps[:, :nw], bt[:, dc : dc + 1],
                                                None, ALU.add)
                    elif on_act:
                        # lead-in: ACT Copy shares the Exp table and shortens
                        # the DVE dep chain in front of the first exps
                        nc.scalar.activation(dst[:, dc, n0 : n0 + nw],
                                             ps[:, :nw], AF.Copy)
                    else:
                        nc.vector.tensor_copy(dst[:, dc, n0 : n0 + nw],
                                              ps[:, :nw])

        def dma_seg(si):
            n0, nw = SEGS[si]
            nc.sync.dma_start(xq_f[:, :, n0 : n0 + nw], xq_v[:, :, n0 : n0 + nw])
            nc.sync.dma_start(x8kv[:, :, n0 : n0 + nw], xkv8_v[:, :, n0 : n0 + nw])
            nc.gpsimd.tensor_copy(x8q[:, :, n0 : n0 + nw],
                                  xq_f[:, :, n0 : n0 + nw])

        def compute_seg(si):
            n0, nw = SEGS[si]
            bag = smls.tile([P, 4, 2, 2], F32, tag="bag", name=f"bag{si}")
            for ci, (mi, m0, mw) in enumerate(_chunks_of_seg(si)):
                norms_for_chunk(mi, m0, mw, ci, bag)
            rd_for_seg(si, bag)
            proj_for_seg(n0, nw, on_act=(si <= 1))

        # ---------------- phase 2 ----------------
        er_tiles = {}

        def s_exp_chunk(sj, mi):
            sn0, snw = SUPERS[sj]
            m0, mw = M_CHUNKS[mi]
            sp = sS.tile([P, 2, SEG], F32, tag="sp", name=f"sp{sj}_{mi}")
            lhsT = k8 if use_bias else x8kv
            rhs = q8 if use_bias else g8
            nh = (snw + SEG - 1) // SEG
            for hi in range(nh):
                hw = min(SEG, snw - hi * SEG)
                nc.tensor.matmul(sp[:mw, hi, :hw], lhsT[:, :, m0 : m0 + mw],
                                 rhs[:, :, sn0 + hi * SEG : sn0 + hi * SEG + hw],
                                 start=True, stop=True, perf_mode=DR)
            pi, slot = mi // 2, mi % 2
            key = (sj, pi)
            if key not in er_tiles:
                if snw > SEG:
                    er_tiles[key] = e8p.tile([P, 2, 2, SEG], F8, tag="er",
                                             name=f"er{sj}_{pi}")
                else:
                    er_tiles[key] = e8s.tile([P, 2, 1, SEG], F8, tag="ers",
                                             name=f"er{sj}_{pi}")
            er = er_tiles[key]
            if snw > SEG:
                nc.scalar.activation(er[:mw, slot, :, :], sp[:mw, :, :], AF.Exp,
                                     scale=rd[:mw, mi : mi + 1])
            else:
                nc.scalar.activation(er[:mw, slot, 0, :snw], sp[:mw, 0, :snw],
                                     AF.Exp, scale=rd[:mw, mi : mi + 1])

        def av_out_sub(sj, s):
                sn0, snw = SUPERS[sj]
                bw = min(P, snw - s * P)
                hh, c0 = s // 4, (s % 4) * P
                acc = aux.tile([P, SEG], F32, tag="aux", name=f"acc{sj}_{s}")
                for pi in range(12):
                    er = er_tiles[(sj, pi)]
                    nc.tensor.matmul(acc[:bw, : C + 2],
                                     er[:, :, hh, c0 : c0 + bw],
                                     pvT8[:, pi, :, :],
                                     start=(pi == 0), stop=False, perf_mode=DR)
                er = er_tiles[(sj, 12)]
                lmw = M_CHUNKS[24][1]
                nc.tensor.matmul(acc[:bw, : C + 2],
                                 er[:lmw, 0, hh, c0 : c0 + bw],
                                 pvT8[:lmw, 12, 0, :],
                                 start=False, stop=True)
                # normalize: un = acc[:, :C] * (1/rowsum), bf16
                rc = smls.tile([P, 1], F32, tag="rc", name=f"rc{sj}_{s}")
                nc.vector.reciprocal(rc[:bw], acc[:bw, C : C + 1])
                un = scrp.tile([P, C], BF16, tag="un", name=f"un{sj}_{s}")
                nc.vector.tensor_scalar(un[:bw], acc[:bw, :C], rc[:bw], None,
                                        ALU.mult)
                pos = sn0 + s * P
                # both c-chunks transpose into ONE psum tile (2nd matmul
                # start=False accumulates into the already-zeroed region),
                # allowing a single fused (tp/16 + xq) STT for the sub
                tp = aux.tile([P, 2, SEG], BF16, tag="aux",
                              name=f"tp{sj}_{s}")
                for cb in range(CC):
                    nc.tensor.matmul(tp[:, cb, :bw],
                                     un[:bw, cb * P : (cb + 1) * P],
                                     ident[:bw, :bw], is_transpose=True,
                                     start=(cb == 0), stop=(cb == CC - 1))
                ot = outp.tile([P, CC, P], F32, tag="ot", name=f"ot{sj}_{s}")
                nc.vector.scalar_tensor_tensor(
                    ot[:, :, :bw], tp[:, :, :bw], 1.0 / WSCALE,
                    xq_f[:, :, pos : pos + bw], ALU.mult, ALU.add)
                if use_bias:
                    for cb in range(CC):
                        nc.vector.tensor_scalar(ot[:, cb, :bw], ot[:, cb, :bw],
                                                bo_sb[:, cb : cb + 1], None,
                                                ALU.add)
                nc.sync.dma_start(out_v[:, :, pos : pos + bw], ot[:, :, :bw])

        def av_out_super(sj):
            snw = SUPERS[sj][1]
            for s in range((snw + P - 1) // P):
                av_out_sub(sj, s)

        # Work-queue emission: an exp for (super sj, chunk mi) is ready once
        # the q8 segs covering the super and the k8/rd seg covering the chunk
        # are computed. Emitting in availability order keeps the ACT queue
        # full from ~seg 2 onward. AV/output subtiles of completed supers are
        # interleaved between exps so the PE queue always has ready work.
        sup_ready_at = [(sn0 + snw - 1) // SEG for sn0, snw in SUPERS]
        n_chunks = len(M_CHUNKS)
        done_chunks = [set() for _ in SUPERS]
        av_pending = []
        av_done = 0
        FILL = 3

        def emit_av(k):
            nonlocal av_done
            while av_done < k and av_done < len(av_pending):
                av_out_sub(*av_pending[av_done])
                av_done += 1

        def emit_exp(sj, mi, av_rate=1):
            if mi in done_chunks[sj]:
                return
            s_exp_chunk(sj, mi)
            done_chunks[sj].add(mi)
            if len(done_chunks[sj]) == n_chunks:
                nsub = (SUPERS[sj][1] + P - 1) // P
                av_pending.extend((sj, s) for s in range(nsub))
            emit_av(av_done + av_rate)

        dma_seg(0)
        preamble()
        for si in range(len(SEGS)):
            if si + 1 < len(SEGS):
                dma_seg(si + 1)
            compute_seg(si)
            for mi2 in PV_PLAN.get(si, ()):
                pv_for_chunk(mi2, *M_CHUNKS[mi2])
            avail = min(4 * (si + 1), n_chunks)
            # Once the last seg lands, the final chunk's exp gates EVERY
            # super's AV: emit all supers' chunk 24 first so AV work can
            # start executing while the remaining exps drain.
            if avail == n_chunks:
                for sj in range(len(SUPERS)):
                    emit_exp(sj, n_chunks - 1)
            # super 0 has priority: finish earlier supers first and fill ACT
            # with just a little of the next super to avoid gaps.
            if sup_ready_at[0] <= si:
                for mi in range(avail):
                    emit_exp(0, mi)
            if si >= 1:
                for sj in range(1, len(SUPERS)):
                    if sup_ready_at[sj] > si or len(done_chunks[sj]) >= avail:
                        continue
                    take = 0
                    for mi in range(avail):
                        if take >= FILL:
                            break
                        if mi not in done_chunks[sj]:
                            emit_exp(sj, mi)
                            take += 1
                    break
        # drain remaining supers, AV interleaved; the tiny last super is
        # drained second-to-last so the final super's exps cover its AV
        for sj in range(1, len(SUPERS)):
            for mi in range(n_chunks):
                emit_exp(sj, mi, av_rate=2)
        emit_av(len(av_pending))

    return nc


_CACHE = {}


def _get_module(use_bias: bool):
    if use_bias not in _CACHE:
        nc = build(use_bias)
        nc.finalize()
        _CACHE[use_bias] = nc
    return _CACHE[use_bias]


def kernel(x_q, x_kv, Wq, bq, Wkv, bkv, Wproj, bproj):
    x_q = np.asarray(x_q, dtype=np.float32)
    x_kv = np.asarray(x_kv, dtype=np.float32)
    Wq = np.asarray(Wq, dtype=np.float32)
    bq = np.asarray(bq, dtype=np.float32)
    Wkv = np.asarray(Wkv, dtype=np.float32)
    bkv = np.asarray(bkv, dtype=np.float32)
    Wproj = np.asarray(Wproj, dtype=np.float32)
    bproj = np.asarray(bproj, dtype=np.float32)

    B, c, H, W = x_q.shape
    assert (c, H * W) == (C, N), (x_q.shape,)
    FP8 = ml_dtypes.float8_e4m3
    xq = np.ascontiguousarray(x_q.reshape(B, C, N))
    xkv8 = np.ascontiguousarray(x_kv.reshape(B, C, N)).astype(FP8)

    Wk = Wkv[:C]
    Wv = Wkv[C:]
    wq8 = np.ascontiguousarray(WSCALE * Wq.T).astype(FP8)
    wk8 = np.ascontiguousarray(WSCALE * Wk.T).astype(FP8)
    w38 = np.ascontiguousarray(WSCALE * (Wproj @ Wv).T).astype(FP8)
    gw8 = np.ascontiguousarray(WSCALE * (Wq.T @ Wk)).astype(FP8)
    bq16 = np.ascontiguousarray(WSCALE * bq)
    bk16 = np.ascontiguousarray(WSCALE * bkv[:C])
    bo = np.ascontiguousarray(Wproj @ bkv[C:] + bproj)

    use_bias = bool(np.any(bq16) or np.any(bk16) or np.any(bo))
    nc = _get_module(use_bias)

    in_maps = [
        {
            "xq": xq[b],
            "xkv8": xkv8[b],
            "wq8": wq8,
            "wk8": wk8,
            "w38": w38,
            "gw8": gw8,
            "bq16": bq16,
            "bk16": bk16,
            "bo": bo,
        }
        for b in range(B)
    ]
    res = run_bass_kernel_spmd(nc, in_maps, core_ids=list(range(B)))
    out = np.stack([res.results[b]["out"] for b in range(B)], axis=0)
    return out.reshape(B, C, H, W)


# revision 4
# speedup vs baseline: 1.1520x; 1.0363x over previous
"""CrossAttention Trainium2 kernel, fp8 DoubleRow edition (1 batch elem/core).

Math per batch element (C=256 channels, N=3136 positions):
    q = Wq xq, k = Wk xkv, pv = (Wproj Wv) xkv          (1x1 convs)
    S[n,m] = q[:,n].k[:,m];  rd[m] = 1/(||q[:,m]|| ||k[:,m]|| + eps)
    A = softmax(S * rd[m] along m);  out = Wproj(A v) + xq residual

Implementation notes:
  * All matmuls in fp8e4 DoubleRow mode (2 fp8 rows/cycle, K=256 in one op).
    Weights and x are scaled by 16 on the host so w entries ~N(0,1) stay out
    of fp8 subnormals. Scales self-cancel in softmax: q,k scale 16 each =>
    S x256, qn2*kn2 x65536, rd = rsqrt(qn2*kn2) absorbs it. pv scale 16 is
    divided out in the output STT.
  * S^T tiles [m=128, n<=1024] (2 psum banks) -> ONE ACT Exp per (chunk,
    n-super) with per-partition scale rd[:,mi], writing fp8 er directly.
    |S*rd| <= ~1.1 so no max-subtraction is needed (exp in [0.3, 3.1]).
  * Norms via transposed DR projections (qT [m,256] psum) + DVE square +
    X-reduce; rd = rsqrt via DVE bit-trick + 2 Newton steps (ACT Sqrt would
    thrash the Exp activation table; gpsimd cannot read PSUM).
  * AV: er (stationary) x pvT|1,1 (moving) accumulating [n-sub, C+2]; the
    ones channels give the softmax row-sum for free. Normalize on DVE,
    bf16 PE transpose back to [c, n], fused (tp/16 + xq) STT, DMA out.
  * PSUM: S pool 2x2 banks + aux pool 4x1 bank (phase-1 qT/kT/pv/proj tiles,
    then AV acc + transpose) = 8 banks exactly.
"""

import sys

if "/opt/trn_rl_repo" not in sys.path:
    sys.path.insert(0, "/opt/trn_rl_repo")

import numpy as np
import ml_dtypes

import concourse.bass as bass
import concourse.mybir as mybir
import concourse.tile as tile
from concourse import bacc
from concourse.bass_utils import run_bass_kernel_spmd
from concourse.masks import make_identity
from contextlib import ExitStack

F32 = mybir.dt.float32
F8 = mybir.dt.float8e4
BF16 = mybir.dt.bfloat16
I32 = mybir.dt.int32
AF = mybir.ActivationFunctionType
DR = mybir.MatmulPerfMode.DoubleRow
ALU = mybir.AluOpType

P = 128
C = 256
CC = C // P            # 2
N = 56 * 56            # 3136
SEG = 512
SEGS = [(i, min(SEG, N - i)) for i in range(0, N, SEG)]            # 7
M_CHUNKS = [(i, min(P, N - i)) for i in range(0, N, P)]            # 25
N_FULL = len(M_CHUNKS) - 1                                         # 24 full
SUPERS = [(0, 1024), (1024, 1024), (2048, 1024), (3072, 64)]
WSCALE = 16.0          # host-side weight/x scale for fp8 range
# pv is needed only once AV starts (after the last seg): keep its psum
# traffic and DVE casts out of the phase-1 production pipeline entirely
PV_PLAN = {6: range(0, 25)}


def _chunks_of_seg(si):
    lo = si * 4
    return [(mi, M_CHUNKS[mi][0], M_CHUNKS[mi][1])
            for mi in range(lo, min(lo + 4, len(M_CHUNKS)))]


def build(use_bias: bool):
    nc = bacc.Bacc(None, target_bir_lowering=False)

    xq_d = nc.dram_tensor("xq", [C, N], F32, kind="ExternalInput")
    xkv8_d = nc.dram_tensor("xkv8", [C, N], F8, kind="ExternalInput")
    wq8_d = nc.dram_tensor("wq8", [C, C], F8, kind="ExternalInput")   # [c, d]
    wk8_d = nc.dram_tensor("wk8", [C, C], F8, kind="ExternalInput")
    w38_d = nc.dram_tensor("w38", [C, C], F8, kind="ExternalInput")
    gw8_d = nc.dram_tensor("gw8", [C, C], F8, kind="ExternalInput")   # 16*Wq.T@Wk
    bq_d = nc.dram_tensor("bq16", [C], F32, kind="ExternalInput")     # 16*bq
    bk_d = nc.dram_tensor("bk16", [C], F32, kind="ExternalInput")     # 16*bk
    bo_d = nc.dram_tensor("bo", [C], F32, kind="ExternalInput")       # Wproj bv + bproj
    out_d = nc.dram_tensor("out", [C, N], F32, kind="ExternalOutput")

    xq_v = xq_d[:].rearrange("(cc p) n -> p cc n", p=P)
    xkv8_v = xkv8_d[:].rearrange("(cc p) n -> p cc n", p=P)
    out_v = out_d[:].rearrange("(cc p) n -> p cc n", p=P)

    with tile.TileContext(nc) as tc, ExitStack() as ctx:
        pers = ctx.enter_context(tc.tile_pool(name="pers", bufs=1))
        sS = ctx.enter_context(tc.tile_pool(name="sS", bufs=2, space="PSUM"))
        aux = ctx.enter_context(tc.tile_pool(name="aux", bufs=4, space="PSUM"))
        scrp = ctx.enter_context(tc.tile_pool(name="scrp", bufs=3))
        e8p = ctx.enter_context(tc.tile_pool(name="e8p", bufs=39))
        e8s = ctx.enter_context(tc.tile_pool(name="e8s", bufs=13))
        outp = ctx.enter_context(tc.tile_pool(name="outp", bufs=4))
        smls = ctx.enter_context(tc.tile_pool(name="smls", bufs=4))

        xq_f = pers.tile([P, CC, N], F32)
        x8q = pers.tile([P, CC, N], F8)
        x8kv = pers.tile([P, CC, N], F8)
        if use_bias:
            q8 = pers.tile([P, CC, N], F8)
            k8 = pers.tile([P, CC, N], F8)
        else:
            # S = xkv^T (Wk^T Wq) xq: one fused projection g = G xq replaces
            # both q and k projections for the S matmul (norms still use
            # the transposed qT/kT products)
            g8 = pers.tile([P, CC, N], F8)
            gw8 = pers.tile([P, CC, C], F8)
        wq8 = pers.tile([P, CC, C], F8)
        wk8 = pers.tile([P, CC, C], F8)
        w38 = pers.tile([P, CC, C], F8)
        pvT8 = pers.tile([P, 13, 2, C + 2], F8)
        rd = pers.tile([P, len(M_CHUNKS)], F32)
        if use_bias:
            qn2 = pers.tile([P, len(M_CHUNKS)], F32)
            kn2 = pers.tile([P, len(M_CHUNKS)], F32)
        ident = pers.tile([P, P], BF16)
        if use_bias:
            bq_sb = pers.tile([P, CC], F32)
            bk_sb = pers.tile([P, CC], F32)
            bo_sb = pers.tile([P, CC], F32)
            bqb = pers.tile([P, C], F32)
            bkb = pers.tile([P, C], F32)

        def preamble():
            make_identity(nc, ident)
            nc.sync.dma_start(wq8, wq8_d[:].rearrange("(cc p) d -> p cc d", p=P))
            nc.sync.dma_start(wk8, wk8_d[:].rearrange("(cc p) d -> p cc d", p=P))
            nc.sync.dma_start(w38, w38_d[:].rearrange("(cc p) d -> p cc d", p=P))
            if not use_bias:
                nc.sync.dma_start(gw8,
                                  gw8_d[:].rearrange("(cc p) d -> p cc d", p=P))
            # softmax row-sum channels
            nc.vector.memset(pvT8[:, :, :, C : C + 2], 1.0)
        if use_bias:
            nc.sync.dma_start(bq_sb, bq_d[:].rearrange("(c p) -> p c", p=P))
            nc.sync.dma_start(bk_sb, bk_d[:].rearrange("(c p) -> p c", p=P))
            nc.sync.dma_start(bo_sb, bo_d[:].rearrange("(c p) -> p c", p=P))
            nc.sync.dma_start(
                bqb, bass.AP(tensor=bq_d[:].tensor, offset=0, ap=[[0, P], [1, C]])
            )
            nc.sync.dma_start(
                bkb, bass.AP(tensor=bk_d[:].tensor, offset=0, ap=[[0, P], [1, C]])
            )

        # ---------------- phase 1 (per 512-seg, pipelined) ----------------
        def norms_for_chunk(mi, m0, mw, ci, bag):
            for side, (which, xsrc, w8) in enumerate(
                (("q", x8q, wq8), ("k", x8kv, wk8))
            ):
                ps = aux.tile([P, SEG], F32, tag="aux", name=f"t{which}{m0}")
                nc.tensor.matmul(ps[:mw, :C], xsrc[:, :, m0 : m0 + mw], w8,
                                 start=True, stop=True, perf_mode=DR)
                if use_bias:
                    nacc = qn2 if which == "q" else kn2
                    bbt = bqb if which == "q" else bkb
                    scr = scrp.tile([P, C], F32, tag="sq", name=f"s{which}{m0}")
                    nc.vector.tensor_add(scr[:mw], ps[:mw, :C], bbt[:mw])
                    nc.vector.scalar_tensor_tensor(
                        scr[:mw], scr[:mw], 1.0, scr[:mw], ALU.mult, ALU.mult,
                        accum_out=nacc[:mw, mi : mi + 1])
                else:
                    # sum(x^2) = n*(var + mean^2) via bn_stats: one DVE pass
                    # over the PSUM tile instead of copy+square+reduce
                    bn6 = scrp.tile([P, 2, 6], F32, tag="bn6",
                                    name=f"b{which}{m0}")
                    nc.vector.bn_stats(bn6[:mw, side, :], ps[:mw, :C])
                    nc.vector.bn_aggr(bag[:mw, ci, side, :], bn6[:mw, side, :])

        def pv_for_chunk(mi, m0, mw):
            ps = aux.tile([P, SEG], F32, tag="aux", name=f"pv{m0}")
            nc.tensor.matmul(ps[:mw, :C], x8kv[:, :, m0 : m0 + mw], w38,
                             start=True, stop=True, perf_mode=DR)
            nc.vector.tensor_copy(pvT8[:mw, mi // 2, mi % 2, :C], ps[:mw, :C])

        def rd_for_seg(si, bag):
            lo = si * 4
            hi = min(lo + 4, len(M_CHUNKS))
            nseg = hi - lo
            u = smls.tile([P, 4], F32, tag="u", name=f"u{si}")
            if use_bias:
                nc.vector.tensor_mul(u[:, :nseg], qn2[:, lo:hi], kn2[:, lo:hi])
            else:
                t = smls.tile([P, 4, 2, 1], F32, tag="tvm", name=f"tvm{si}")
                mean = bag[:, :nseg, :, 0:1]
                var = bag[:, :nseg, :, 1:2]
                nc.vector.scalar_tensor_tensor(t[:, :nseg], mean, 1.0, mean,
                                               ALU.mult, ALU.mult)
                nc.vector.tensor_add(t[:, :nseg], t[:, :nseg], var)
                nc.vector.tensor_mul(u[:, :nseg], t[:, :nseg, 0, 0],
                                     t[:, :nseg, 1, 0])
            yb = smls.tile([P, 4], I32, tag="yb", name=f"yb{si}")
            nc.vector.tensor_scalar(yb[:, :nseg], u[:, :nseg].bitcast(I32),
                                    1, None, ALU.logical_shift_right)
            nc.vector.tensor_scalar(yb[:, :nseg], yb[:, :nseg], -1, 0x5F3759DF,
                                    ALU.mult, ALU.add)
            y = yb.bitcast(F32)
            h = smls.tile([P, 4], F32, tag="h", name=f"h{si}")
            for _ in range(2):
                nc.vector.tensor_mul(h[:, :nseg], y[:, :nseg], y[:, :nseg])
                nc.vector.tensor_mul(h[:, :nseg], h[:, :nseg], u[:, :nseg])
                nc.vector.tensor_scalar(h[:, :nseg], h[:, :nseg], -0.5, 1.5,
                                        ALU.mult, ALU.add)
                nc.vector.tensor_mul(y[:, :nseg], y[:, :nseg], h[:, :nseg])
            if use_bias:
                nc.vector.tensor_copy(rd[:, lo:hi], y[:, :nseg])
            else:
                # g-fold S is 16*S; u = (qn*kn/256)^2 => rd = rsqrt(u)/16
                nc.vector.tensor_scalar(rd[:, lo:hi], y[:, :nseg], 1.0 / 16.0,
                                        None, ALU.mult)

        def proj_for_seg(n0, nw, on_act=False):
            if use_bias:
                plan = (("q", x8q, wq8, q8, bq_sb), ("k", x8kv, wk8, k8, bk_sb))
            else:
                plan = (("g", x8q, gw8, g8, None),)
            for which, xsrc, w8, dst, bt in plan:
                for dc in range(CC):
                    ps = aux.tile([P, SEG], F32, tag="aux",
                                  name=f"p{which}{n0}_{dc}")
                    nc.tensor.matmul(ps[:, :nw], w8[:, :, dc * P : (dc + 1) * P],
                                     xsrc[:, :, n0 : n0 + nw],
                                     start=True, stop=True, perf_mode=DR)
                    if use_bias:
                        nc.vector.tensor_scalar(dst[:, dc, n0 : n0 + nw],
                                                ps[:, :nw], bt[:, dc : dc + 1],
                                                None, ALU.add)
                    elif on_act:
                        # lead-in: ACT Copy shares the Exp table and shortens
                        # the DVE dep chain in front of the first exps
                        nc.scalar.activation(dst[:, dc, n0 : n0 + nw],
                                             ps[:, :nw], AF.Copy)
                    else:
                        nc.vector.tensor_copy(dst[:, dc, n0 : n0 + nw],
                                              ps[:, :nw])

        def dma_seg(si):
            n0, nw = SEGS[si]
            nc.sync.dma_start(xq_f[:, :, n0 : n0 + nw], xq_v[:, :, n0 : n0 + nw])
            nc.sync.dma_start(x8kv[:, :, n0 : n0 + nw], xkv8_v[:, :, n0 : n0 + nw])
            # two half-casts so the first chunks' norm matmuls (and the
            # first S matmuls) unblock ~0.7us earlier per seg
            h1 = min(256, nw)
            nc.gpsimd.tensor_copy(x8q[:, :, n0 : n0 + h1],
                                  xq_f[:, :, n0 : n0 + h1])
            if nw > h1:
                nc.gpsimd.tensor_copy(x8q[:, :, n0 + h1 : n0 + nw],
                                      xq_f[:, :, n0 + h1 : n0 + nw])

        def compute_seg(si):
            n0, nw = SEGS[si]
            bag = smls.tile([P, 4, 2, 2], F32, tag="bag", name=f"bag{si}")
            for ci, (mi, m0, mw) in enumerate(_chunks_of_seg(si)):
                norms_for_chunk(mi, m0, mw, ci, bag)
            rd_for_seg(si, bag)
            proj_for_seg(n0, nw, on_act=(si <= 1))

        # ---------------- phase 2 ----------------
        er_tiles = {}

        def s_exp_chunk(sj, mi):
            sn0, snw = SUPERS[sj]
            m0, mw = M_CHUNKS[mi]
            sp = sS.tile([P, 2, SEG], F32, tag="sp", name=f"sp{sj}_{mi}")
            lhsT = k8 if use_bias else x8kv
            rhs = q8 if use_bias else g8
            nh = (snw + SEG - 1) // SEG
            for hi in range(nh):
                hw = min(SEG, snw - hi * SEG)
                nc.tensor.matmul(sp[:mw, hi, :hw], lhsT[:, :, m0 : m0 + mw],
                                 rhs[:, :, sn0 + hi * SEG : sn0 + hi * SEG + hw],
                                 start=True, stop=True, perf_mode=DR)
            pi, slot = mi // 2, mi % 2
            key = (sj, pi)
            if key not in er_tiles:
                if snw > SEG:
                    er_tiles[key] = e8p.tile([P, 2, 2, SEG], F8, tag="er",
                                             name=f"er{sj}_{pi}")
                else:
                    er_tiles[key] = e8s.tile([P, 2, 1, SEG], F8, tag="ers",
                                             name=f"er{sj}_{pi}")
            er = er_tiles[key]
            if snw > SEG:
                nc.scalar.activation(er[:mw, slot, :, :], sp[:mw, :, :], AF.Exp,
                                     scale=rd[:mw, mi : mi + 1])
            else:
                nc.scalar.activation(er[:mw, slot, 0, :snw], sp[:mw, 0, :snw],
                                     AF.Exp, scale=rd[:mw, mi : mi + 1])

        av_mid_q = []
        av_back_q = []

        def av_flush_back():
            while av_mid_q:
                av_mid_q.pop(0)()
            while av_back_q:
                av_back_q.pop(0)()

        def av_out_sub(sj, s):
                sn0, snw = SUPERS[sj]
                bw = min(P, snw - s * P)
                hh, c0 = s // 4, (s % 4) * P
                acc = aux.tile([P, SEG], F32, tag="aux", name=f"acc{sj}_{s}")
                for pi in range(12):
                    er = er_tiles[(sj, pi)]
                    nc.tensor.matmul(acc[:bw, : C + 2],
                                     er[:, :, hh, c0 : c0 + bw],
                                     pvT8[:, pi, :, :],
                                     start=(pi == 0), stop=False, perf_mode=DR)
                er = er_tiles[(sj, 12)]
                lmw = M_CHUNKS[24][1]
                nc.tensor.matmul(acc[:bw, : C + 2],
                                 er[:lmw, 0, hh, c0 : c0 + bw],
                                 pvT8[:lmw, 12, 0, :],
                                 start=False, stop=True)
                rc = smls.tile([P, 1], F32, tag="rc", name=f"rc{sj}_{s}")
                nc.vector.reciprocal(rc[:bw], acc[:bw, C : C + 1])
                un = scrp.tile([P, C], BF16, tag="un", name=f"un{sj}_{s}")
                nc.vector.tensor_scalar(un[:bw], acc[:bw, :C], rc[:bw], None,
                                        ALU.mult)
                pos = sn0 + s * P

                def mid(un=un, bw=bw, sj=sj, s=s, pos=pos):
                    # both c-chunks transpose into ONE psum tile (2nd matmul
                    # start=False accumulates into the already-zeroed region)
                    tp = aux.tile([P, 2, SEG], BF16, tag="aux",
                                  name=f"tp{sj}_{s}")
                    for cb in range(CC):
                        nc.tensor.matmul(tp[:, cb, :bw],
                                         un[:bw, cb * P : (cb + 1) * P],
                                         ident[:bw, :bw], is_transpose=True,
                                         start=(cb == 0), stop=(cb == CC - 1))

                    def back():
                        ot = outp.tile([P, CC, P], F32, tag="ot",
                                       name=f"ot{sj}_{s}")
                        nc.vector.scalar_tensor_tensor(
                            ot[:, :, :bw], tp[:, :, :bw], 1.0 / WSCALE,
                            xq_f[:, :, pos : pos + bw], ALU.mult, ALU.add)
                        if use_bias:
                            for cb in range(CC):
                                nc.vector.tensor_scalar(ot[:, cb, :bw],
                                                        ot[:, cb, :bw],
                                                        bo_sb[:, cb : cb + 1],
                                                        None, ALU.add)
                        nc.sync.dma_start(out_v[:, :, pos : pos + bw],
                                          ot[:, :, :bw])

                    av_back_q.append(back)

                # stage the PE transposes one sub behind the AV matmuls and
                # the DVE output STT two behind, so neither engine's FIFO
                # ever stalls on a cross-engine round-trip
                av_mid_q.append(mid)
                if len(av_mid_q) > 1:
                    av_mid_q.pop(0)()
                if len(av_back_q) > 1:
                    av_back_q.pop(0)()

        def av_out_super(sj):
            snw = SUPERS[sj][1]
            for s in range((snw + P - 1) // P):
                av_out_sub(sj, s)

        # Work-queue emission: an exp for (super sj, chunk mi) is ready once
        # the q8 segs covering the super and the k8/rd seg covering the chunk
        # are computed. Emitting in availability order keeps the ACT queue
        # full from ~seg 2 onward. AV/output subtiles of completed supers are
        # interleaved between exps so the PE queue always has ready work.
        sup_ready_at = [(sn0 + snw - 1) // SEG for sn0, snw in SUPERS]
        n_chunks = len(M_CHUNKS)
        done_chunks = [set() for _ in SUPERS]
        av_pending = []
        av_done = 0
        FILL = 2

        def emit_av(k):
            nonlocal av_done
            while av_done < k and av_done < len(av_pending):
                av_out_sub(*av_pending[av_done])
                av_done += 1

        def emit_exp(sj, mi, av_rate=1):
            if mi in done_chunks[sj]:
                return
            s_exp_chunk(sj, mi)
            done_chunks[sj].add(mi)
            if len(done_chunks[sj]) == n_chunks:
                nsub = (SUPERS[sj][1] + P - 1) // P
                av_pending.extend((sj, s) for s in range(nsub))
            emit_av(av_done + av_rate)

        dma_seg(0)
        preamble()
        for si in range(len(SEGS)):
            if si + 1 < len(SEGS):
                dma_seg(si + 1)
            compute_seg(si)
            for mi2 in PV_PLAN.get(si, ()):
                pv_for_chunk(mi2, *M_CHUNKS[mi2])
            avail = min(4 * (si + 1), n_chunks)
            # Once the last seg lands, the final chunk's exp gates EVERY
            # super's AV: emit all supers' chunk 24 first so AV work can
            # start executing while the remaining exps drain.
            if avail == n_chunks:
                for sj in range(len(SUPERS)):
                    emit_exp(sj, n_chunks - 1)
            # super 0 has priority: finish earlier supers first and fill ACT
            # with just a little of the next super to avoid gaps.
            if sup_ready_at[0] <= si:
                for mi in range(avail):
                    emit_exp(0, mi)
            if si >= 1:
                for sj in range(1, len(SUPERS)):
                    if sup_ready_at[sj] > si or len(done_chunks[sj]) >= avail:
                        continue
                    take = 0
                    for mi in range(avail):
                        if take >= FILL:
                            break
                        if mi not in done_chunks[sj]:
                            emit_exp(sj, mi)
                            take += 1
                    break
        # drain remaining supers, AV interleaved; the tiny last super is
        # drained second-to-last so the final super's exps cover its AV
        # Drain order: super 1, most of super 2, super 3, then the held-back
        # tail of super 2. Super 3's AV (gated by its last exp) then overlaps
        # the held-back window, and super 2's AV overlaps its own tail exps.
        for sj in range(1, len(SUPERS)):
            for mi in range(n_chunks):
                emit_exp(sj, mi, av_rate=1)
        emit_av(len(av_pending))
        av_flush_back()

    return nc


_CACHE = {}


def _get_module(use_bias: bool):
    if use_bias not in _CACHE:
        nc = build(use_bias)
        nc.finalize()
        _CACHE[use_bias] = nc
    return _CACHE[use_bias]


def kernel(x_q, x_kv, Wq, bq, Wkv, bkv, Wproj, bproj):
    x_q = np.asarray(x_q, dtype=np.float32)
    x_kv = np.asarray(x_kv, dtype=np.float32)
    Wq = np.asarray(Wq, dtype=np.float32)
    bq = np.asarray(bq, dtype=np.float32)
    Wkv = np.asarray(Wkv, dtype=np.float32)
    bkv = np.asarray(bkv, dtype=np.float32)
    Wproj = np.asarray(Wproj, dtype=np.float32)
    bproj = np.asarray(bproj, dtype=np.float32)

    B, c, H, W = x_q.shape
    assert (c, H * W) == (C, N), (x_q.shape,)
    FP8 = ml_dtypes.float8_e4m3
    xq = np.ascontiguousarray(x_q.reshape(B, C, N))
    xkv8 = np.ascontiguousarray(x_kv.reshape(B, C, N)).astype(FP8)

    Wk = Wkv[:C]
    Wv = Wkv[C:]
    wq8 = np.ascontiguousarray(WSCALE * Wq.T).astype(FP8)
    wk8 = np.ascontiguousarray(WSCALE * Wk.T).astype(FP8)
    w38 = np.ascontiguousarray(WSCALE * (Wproj @ Wv).T).astype(FP8)
    gw8 = np.ascontiguousarray(WSCALE * (Wq.T @ Wk)).astype(FP8)
    bq16 = np.ascontiguousarray(WSCALE * bq)
    bk16 = np.ascontiguousarray(WSCALE * bkv[:C])
    bo = np.ascontiguousarray(Wproj @ bkv[C:] + bproj)

    use_bias = bool(np.any(bq16) or np.any(bk16) or np.any(bo))
    nc = _get_module(use_bias)

    in_maps = [
        {
            "xq": xq[b],
            "xkv8": xkv8[b],
            "wq8": wq8,
            "wk8": wk8,
            "w38": w38,
            "gw8": gw8,
            "bq16": bq16,
            "bk16": bk16,
            "bo": bo,
        }
        for b in range(B)
    ]
    res = run_bass_kernel_spmd(nc, in_maps, core_ids=list(range(B)))
    out = np.stack([res.results[b]["out"] for b in range(B)], axis=0)
    return out.reshape(B, C, H, W)
